# revision 72
# baseline (speedup 1.0000x reference)
"""Multi-head attention Trainium2 kernel, 8-core SPMD.

Sharding: 16 (batch, head) pairs over 8 cores -> each core computes 2 heads
of one batch and returns a partial [N, D] output; host sums 4 partials per
batch.

Per-core dataflow (all layouts transposed, q/m on free dims so softmax'
normalization can be deferred):
  XT = dma-transpose(x)                [D, N]  bf16, chunked + split across
                                       both HWDGE rings (sync + act) so the
                                       first projection starts ~4us in
  QT/KT = W.T @ XT                     [2*HS, N] per head pair (scale folded
                                       into Wq on host)
  Vnat  = XT_chunk.T @ Wv              [m, 2*HS] per m-chunk (natural layout
                                       directly; no PE transposes)
  S^T[m,q] = KT_h.T @ QT_h             PSUM fp32, per m-chunk of 128
  P^T = exp(S^T)                       ACT, -> SBUF bf16 (no max subtraction:
                                       logits are O(6) by construction)
  O^T[65,q] = [V_h | 1].T @ P^T        PSUM accumulate over m; row 64 = row
                                       sums r[q] (ones-column trick)
  U = O^T -> SBUF; Un = U[0:64] / r    (recip + partition broadcast)
  out[q,:] += Un_h.T @ Wp_h            accumulated over both heads in PSUM
"""

import os
import sys

import numpy as np

sys.path.insert(0, "/opt/trn_rl_repo")

import ml_dtypes
from contextlib import ExitStack

import concourse.bass as bass
import concourse.mybir as mybir
import concourse.tile as tile
from concourse import bacc
from concourse.bass_utils import run_bass_kernel_spmd

B, N, D, H, HS = 2, 2048, 512, 8, 64
NCORES = 8
BF16 = mybir.dt.bfloat16
FP32 = mybir.dt.float32
nbf16 = ml_dtypes.bfloat16

DC = D // 128  # 4 d-chunks
MC = N // 128  # 16 m-chunks
QH = 2  # q halves
QW = N // QH  # 1024 q per chunk


def build_nc(finalize=True):
    nc = bacc.Bacc()
    # x tensors arrive pre-transposed [D, N] and w tensors pre-swizzled to
    # the on-chip [128, dc*128] layout (host-side prep), so every input DMA
    # is a big contiguous transfer - no xbar transposes (which the tile
    # scheduler serializes against all other DMAs)
    xq = nc.dram_tensor("xq", [D, N], BF16, kind="ExternalInput")
    xk = nc.dram_tensor("xk", [D, N], BF16, kind="ExternalInput")
    xv = nc.dram_tensor("xv", [D, N], BF16, kind="ExternalInput")
    # all weights in one tensor = one DMA trigger: [p, 4, dc*128] holding
    # wq|wk|wv (swizzled) and wp
    wall = nc.dram_tensor("wall", [128, 4, DC * 128], BF16, kind="ExternalInput")
    out = nc.dram_tensor("out", [N, D], FP32, kind="ExternalOutput")

    with tile.TileContext(nc) as tc, ExitStack() as ctx:
        consts = ctx.enter_context(tc.tile_pool(name="consts", bufs=1))
        xt_pool = ctx.enter_context(tc.tile_pool(name="xt", bufs=1))
        proj_pool = ctx.enter_context(tc.tile_pool(name="proj", bufs=1))
        pt_pool = ctx.enter_context(tc.tile_pool(name="pt", bufs=14))
        u_pool = ctx.enter_context(tc.tile_pool(name="u", bufs=4))
        un_pool = ctx.enter_context(tc.tile_pool(name="un", bufs=4))
        rb_pool = ctx.enter_context(tc.tile_pool(name="rb", bufs=2))
        ob_pool = ctx.enter_context(tc.tile_pool(name="ob", bufs=3))
        psA = ctx.enter_context(tc.tile_pool(name="psA", bufs=2, space="PSUM"))
        psO = ctx.enter_context(tc.tile_pool(name="psO", bufs=2, space="PSUM"))

        # ---- weights: one DMA on the act HWDGE ring ----
        wall_s = consts.tile([128, 4, DC, 128], BF16, tag="wall_s")
        wq_s = wall_s[:, 0]
        wk_s = wall_s[:, 1]
        wv_s = wall_s[:, 2]
        wp_s = wall_s[:, 3].rearrange("p c h -> p (c h)")
        # Vnat: [128m, mc, head, 65]; col 64 = ones (rowsum trick)
        vnat = consts.tile([128, MC, 2, HS + 1], BF16, tag="vnat")
        # lhsT/rhs must share a base partition; the rowsum row lives at
        # partition HS, so put the ones row there too
        ones_row = consts.tile([HS + 1, HS], BF16, tag="ones_row")
        # scratch for PE warm-up filler matmuls (flips the HAM clock gate to
        # 2.4 GHz before the S stream starts, while waiting on the q DMA);
        # memset first so the PE can start as early as possible
        fill_sb = consts.tile([128, 1024], BF16, tag="fill_sb")
        nc.gpsimd.memset(fill_sb[:], 0.125)
        nc.gpsimd.memset(vnat[:, :, :, HS : HS + 1], 1.0)
        nc.gpsimd.memset(ones_row[HS : HS + 1, :], 1.0)
        # preload the Exp table during the preamble so the first real exp
        # doesn't pay the lazy ACT_TABLE_LOAD (~1.3us)
        tbl_warm = consts.tile([1, 1], BF16, tag="tbl_warm")
        nc.scalar.activation(
            tbl_warm[:], fill_sb[0:1, 0:1], mybir.ActivationFunctionType.Exp
        )

        def emit_fill(n, pool=None, tag="ps"):
            # n x 512-column matmuls with no data dependencies: PE activity
            # filler that holds the HAM clock gate at 8/8
            for _f in range(n):
                fps = (pool or psA).tile(
                    [128, 512], FP32, tag=tag, name="fill_ps"
                )
                nc.tensor.matmul(
                    fps[:], fill_sb[:, 0:128], fill_sb[:, 0:512],
                    start=True, stop=True,
                )

        def emit_ldw_fill(n):
            # PSUM-free PE-array activity (weight loads of a ready constant):
            # can never stall the queue, keeps the HAM clock gate warm
            # through thin stretches
            for _f in range(n):
                nc.tensor.ldweights(fill_sb[:, 0:128])

        # ---- X transposed: [128, dc, N] per tensor, plain chunked DMAs
        # (hosts sends X pre-transposed), k/v on the sync ring, q on the act
        # ring, ordered by first use so compute starts ~2us after the
        # engine preamble
        xts = {}
        for name in ("q", "k", "v"):
            xts[name] = xt_pool.tile(
                [128, DC, N], BF16, tag=f"xt_{name}", name=f"xt_{name}"
            )

        def emit_xload(eng, name, lo, hi):
            dram = {"q": xq, "k": xk, "v": xv}[name]
            eng.dma_start(
                out=xts[name][:, :, lo:hi],
                in_=dram.rearrange("(c p) n -> p c n", p=128)[:, :, lo:hi],
            )

        wall_r = wall.rearrange("p f (c h) -> p f c h", h=128)
        nc.scalar.dma_start(out=wall_s[:, 0:2], in_=wall_r[:, 0:2])
        emit_xload(nc.scalar, "q", 0, 1024)
        nc.scalar.dma_start(out=wall_s[:, 2:4], in_=wall_r[:, 2:4])
        emit_xload(nc.scalar, "q", 1024, 2048)
        emit_xload(nc.sync, "k", 0, 512)
        emit_xload(nc.sync, "k", 512, 1024)
        emit_xload(nc.sync, "v", 0, 1024)
        emit_xload(nc.sync, "k", 1024, 1536)
        emit_xload(nc.sync, "k", 1536, 2048)
        emit_xload(nc.sync, "v", 1024, 2048)

        # ---- projections ----
        wmap = {"q": wq_s, "k": wk_s}
        projT = {}
        for name in ("q", "k"):
            projT[name] = proj_pool.tile(
                [128, N], BF16, tag=f"projT_{name}", name=f"projT_{name}"
            )

        def emit_proj(name, lo, hi):
            # [2*HS, lo:hi] = sum_dc W[dc].T @ XT[dc, lo:hi]
            w = hi - lo
            ps = psA.tile([128, w], FP32, tag="ps", name="ps")
            for s0 in range(0, w, 512):
                sw = min(512, w - s0)
                for dc in range(DC):
                    nc.tensor.matmul(
                        ps[:, s0 : s0 + sw],
                        wmap[name][:, dc, :],
                        xts[name][:, dc, lo + s0 : lo + s0 + sw],
                        start=(dc == 0),
                        stop=(dc == DC - 1),
                    )
            nc.vector.tensor_copy(projT[name][:, lo:hi], ps[:])

        def emit_vnat(g):
            # natural-layout V for m-chunks 4g..4g+3: [128m, 128hs2] each
            ps = psA.tile([128, 512], FP32, tag="ps", name="vg_ps")
            for i in range(4):
                m0 = (4 * g + i) * 128
                for dc in range(DC):
                    nc.tensor.matmul(
                        ps[:, i * 128 : (i + 1) * 128],
                        xts["v"][:, dc, m0 : m0 + 128],
                        wv_s[:, dc, :],
                        start=(dc == 0),
                        stop=(dc == DC - 1),
                    )
            nc.vector.tensor_copy(
                vnat[:, 4 * g : 4 * (g + 1), :, 0:HS],
                ps[:].rearrange("p (m b c) -> p m b c", m=4, b=2),
            )

        # pre-loop: HAM warm-up fillers run while the k/q DMAs are still in
        # flight (PE busy 3.4us+ flips the clock gate), then just enough
        # projection for the first S chunks, with more fillers woven in to
        # bridge the DMA-gated stretch up to the first S without idling
        for _f in range(15):
            fps = psO.tile([HS, QW], FP32, tag="o", name="fill_ps")
            for sl in range(QW // 512):
                nc.tensor.matmul(
                    fps[:, sl * 512 : (sl + 1) * 512],
                    fill_sb[:, 0:HS],
                    fill_sb[:, sl * 512 : (sl + 1) * 512],
                    start=True,
                    stop=True,
                )
        emit_proj("k", 0, 512)
        emit_proj("q", 0, 1024)

        # deferred work, paced into the first m-loop at given (mc, hh) slots
        # so the PE never stalls on a DMA that hasn't landed yet
        deferred = {
            (1, 0): lambda: emit_proj("k", 512, 1024),
            (2, 0): lambda: emit_vnat(0),
            (3, 0): lambda: emit_proj("k", 1024, 1536),
            (4, 0): lambda: emit_proj("k", 1536, 2048),
            (5, 0): lambda: emit_vnat(1),
            (6, 0): lambda: emit_proj("q", 1024, 2048),
            (8, 0): lambda: emit_vnat(2),
            (9, 0): lambda: emit_vnat(3),
        }
        trickle = []  # qh0's normalization/final chain, run during qh1 loop
        carry = []  # qh0's last PVs, replayed through qh1's ramp units

        # attention + output projection - both heads' m-loops
        # interleaved so the PE stream stays dense (holds HAM warm)
        qt2, kt2 = projT["q"], projT["k"]
        for qh in range(QH):
            un2 = un_pool.tile([128, QW], BF16, tag="un")
            o_ps = {}

            def alloc_o():
                for hh in range(2):
                    o_ps[hh] = psO.tile(
                        [HS + 1, QW], FP32, tag="o", name=f"o_ps{hh}"
                    )



            def pv(hh, j, p_sb, o_ps=o_ps):
                for sl in range(QW // 512):
                    nc.tensor.matmul(
                        o_ps[hh][:, sl * 512 : (sl + 1) * 512],
                        vnat[:, j, hh, :],
                        p_sb[:, sl * 512 : (sl + 1) * 512],
                        start=(j == 0),
                        stop=(j == MC - 1),
                    )

            pend = []
            for mc in range(MC):
                for hh in range(2):
                    hs0 = HS * hh
                    s_ps = psA.tile([128, QW], FP32, tag="ps", name="s_ps")
                    for sl in range(QW // 512):
                        nc.tensor.matmul(
                            s_ps[:, sl * 512 : (sl + 1) * 512],
                            kt2[hs0 : hs0 + HS, mc * 128 : (mc + 1) * 128],
                            qt2[
                                hs0 : hs0 + HS,
                                qh * QW + sl * 512 : qh * QW + (sl + 1) * 512,
                            ],
                            start=True,
                            stop=True,
                        )
                    p_sb = pt_pool.tile([128, QW], BF16, tag="p", name="p_sb")
                    nc.scalar.activation(
                        p_sb[:], s_ps[:], mybir.ActivationFunctionType.Exp
                    )
                    if qh == 0:
                        if mc < 3:
                            # the PE is s_ps-slot-limited while the ACT
                            # pipeline ramps: pad its duty so the clock
                            # gate holds 8/8 (o_ps not yet allocated, so
                            # the psO slots are free)
                            emit_fill(1, pool=psO, tag="o")
                        if (mc, hh) == (3, 0):
                            alloc_o()
                        fn = deferred.pop((mc, hh), None)
                        if fn is not None:
                            fn()
                    else:
                        if carry:
                            # replay a ready PV from the previous half:
                            # keeps PE duty high through this half's ramp
                            cpv, e = carry.pop(0)
                            cpv(*e)
                        if trickle and not carry:
                            trickle.pop(0)()
                        if (mc, hh) == (3, 0):
                            alloc_o()
                    pend.append((hh, mc, p_sb))
                    if qh == 0:
                        # hold the last 8 PVs; they replay through the next
                        # half's ramp units to keep PE duty high
                        lag = 6 if mc < 14 else 8
                    else:
                        lag = 9 if mc < 7 else (6 if mc < MC - 1 else 2)
                    while len(pend) > lag:
                        pv(*pend.pop(0))

            def emit_ucopy(hh, o_ps_=None, ueng=None, lo=0, w=QW):
                # PSUM -> SBUF stage; frees the o_ps slot for the next
                # q-half's PV accumulation
                o_ps_ = o_ps_ if o_ps_ is not None else o_ps
                u = u_pool.tile([HS + 1, w], BF16, tag="u", name="u")
                if ueng is None:
                    nc.vector.tensor_copy(u[:], o_ps_[hh][:, lo : lo + w])
                else:
                    ueng.copy(u[:], o_ps_[hh][:, lo : lo + w])
                return u

            def emit_norm(hh, u, un2_, lo=0, w=QW):
                # broadcast row sums r to 64 partitions via ones.T @ r
                # (rb reuses a freed o_ps slot)
                rb_ps = psO.tile([HS, w], FP32, tag="o", name="rb_ps")
                for sl in range(w // 512):
                    nc.tensor.matmul(
                        rb_ps[:, sl * 512 : (sl + 1) * 512],
                        ones_row[HS : HS + 1, :],
                        u[HS : HS + 1, sl * 512 : (sl + 1) * 512],
                        start=True,
                        stop=True,
                    )
                rb = rb_pool.tile([HS, w], FP32, tag="rb", name="rb")
                nc.vector.reciprocal_approx_fast(rb[:], rb_ps[:])
                nc.vector.tensor_mul(
                    un2_[HS * hh : HS * hh + HS, lo : lo + w], u[0:HS, :], rb[:]
                )

            def emit_uchain(hh, o_ps_, un2_):
                u = emit_ucopy(hh, o_ps_)
                emit_norm(hh, u, un2_)

            def emit_final(qh_, un2_, c, act_copy=False):
                f_ps = psA.tile([128, D], FP32, tag="ps", name="f_ps")
                nc.tensor.matmul(
                    f_ps[:],
                    un2_[:, c * 128 : (c + 1) * 128],
                    wp_s[:],
                    start=True,
                    stop=True,
                )
                ob = ob_pool.tile([128, D], FP32, tag="ob", name="ob")
                if act_copy:
                    nc.scalar.copy(ob[:], f_ps[:])
                    deng = nc.scalar
                else:
                    nc.vector.tensor_copy(ob[:], f_ps[:])
                    deng = nc.sync
                deng.dma_start(
                    out=out[qh_ * QW + c * 128 : qh_ * QW + (c + 1) * 128, :],
                    in_=ob[:],
                )

            if qh < QH - 1:
                carry.extend((pv, e) for e in pend)
                pend = []
                for hh in range(2):
                    trickle.append(
                        lambda hh_=hh, o_=o_ps, u_=un2: emit_uchain(hh_, o_, u_)
                    )
                for c in range(QW // 128):
                    trickle.append(
                        lambda qh_=qh, un2_=un2, c_=c: emit_final(qh_, un2_, c_)
                    )
            else:
                # staggered tail: both last PVs first on the PE queue, then
                # per-512-column chains with h1's u copies on the (now idle)
                # ACT engine; finals start as soon as their column's chains
                # are done; copies and out-DMAs alternate ACT/sync rings
                # h0's last PVs first so its u copies (DVE) overlap h1's
                # final PVs on the PE; then h1's copies on ACT
                for e in [e for e in pend if e[0] == 0]:
                    pv(*e)
                uas = [emit_ucopy(0, lo=c * 512, w=512) for c in range(2)]
                for e in [e for e in pend if e[0] == 1]:
                    pv(*e)
                ubs = [
                    emit_ucopy(1, ueng=nc.scalar, lo=c * 512, w=512)
                    for c in range(2)
                ]
                us = list(zip(uas, ubs))
                for col in range(2):
                    lo = col * 512
                    ua, ub = us[col]
                    emit_norm(0, ua, un2, lo=lo, w=512)
                    emit_norm(1, ub, un2, lo=lo, w=512)
                    for c in range(4 * col, 4 * col + 4):
                        emit_final(qh, un2, c, act_copy=(c % 2 == 1))
    if finalize:
        nc.finalize()
    return nc


_NC_CACHE = None


def _get_nc():
    global _NC_CACHE
    if _NC_CACHE is None:
        _NC_CACHE = build_nc()
    return _NC_CACHE


def make_in_maps(inputs):
    query = np.asarray(inputs["query"], np.float32)
    key = np.asarray(inputs["key"], np.float32)
    value = np.asarray(inputs["value"], np.float32)
    Wq = np.asarray(inputs["Wq"], np.float32) / np.sqrt(np.float32(HS))
    Wk = np.asarray(inputs["Wk"], np.float32)
    Wv = np.asarray(inputs["Wv"], np.float32)
    Wp = np.asarray(inputs["Wp"], np.float32)

    def wprep(w, h0):
        # [D, 128] head-pair matrix -> on-chip [128, dc*128] layout where
        # row p, col (c*128+h) holds w[c*128+p, h]
        ww = np.concatenate([w[h0], w[h0 + 1]], axis=1)  # [D, 128]
        return ww.reshape(DC, 128, 128).transpose(1, 0, 2).reshape(128, DC * 128)

    xqT = [np.ascontiguousarray(query[b].T).astype(nbf16) for b in range(B)]
    xkT = [np.ascontiguousarray(key[b].T).astype(nbf16) for b in range(B)]
    xvT = [np.ascontiguousarray(value[b].T).astype(nbf16) for b in range(B)]

    in_maps = []
    for c in range(NCORES):
        b = c // 4
        h0 = 2 * (c % 4)
        wall = np.stack(
            [
                wprep(Wq, h0),
                wprep(Wk, h0),
                wprep(Wv, h0),
                np.concatenate([Wp[h0], Wp[h0 + 1]], axis=0),
            ],
            axis=1,
        )  # [128, 4, dc*128]
        in_maps.append(
            {
                "xq": xqT[b],
                "xk": xkT[b],
                "xv": xvT[b],
                "wall": np.ascontiguousarray(wall).astype(nbf16),
            }
        )
    return in_maps


def kernel(query, key, value, Wq, Wk, Wv, Wp):
    in_maps = make_in_maps(
        dict(query=query, key=key, value=value, Wq=Wq, Wk=Wk, Wv=Wv, Wp=Wp)
    )
    nc = _get_nc()
    res = run_bass_kernel_spmd(nc, in_maps, list(range(NCORES)))
    out = np.zeros((B, N, D), np.float32)
    for c in range(NCORES):
        out[c // 4] += np.asarray(res.results[c]["out"], np.float32)
    return out


if __name__ == "__main__":
    d = np.load("/root/problem/work/ref.npz")
    got = kernel(
        d["query"], d["key"], d["value"], d["Wq"], d["Wk"], d["Wv"], d["Wp"]
    )
    exp = d["expected"]
    rel = np.linalg.norm(got - exp) / np.linalg.norm(exp)
    print("Relative error:", rel)


# revision 73
# speedup vs baseline: 1.1926x; 1.1926x over previous
"""Multi-head attention Trainium2 kernel, 8-core SPMD.

Sharding: 16 (batch, head) pairs over 8 cores -> each core computes 2 heads
of one batch and returns a partial [N, D] output; host sums 4 partials per
batch.

Per-core dataflow (all layouts transposed, q/m on free dims so softmax'
normalization can be deferred):
  XT = dma-transpose(x)                [D, N]  bf16, chunked + split across
                                       both HWDGE rings (sync + act) so the
                                       first projection starts ~4us in
  QT/KT = W.T @ XT                     [2*HS, N] per head pair (scale folded
                                       into Wq on host)
  Vnat  = XT_chunk.T @ Wv              [m, 2*HS] per m-chunk (natural layout
                                       directly; no PE transposes)
  S^T[m,q] = KT_h.T @ QT_h             PSUM fp32, per m-chunk of 128
  P^T = exp(S^T)                       ACT, -> SBUF bf16 (no max subtraction:
                                       logits are O(6) by construction)
  O^T[65,q] = [V_h | 1].T @ P^T        PSUM accumulate over m; row 64 = row
                                       sums r[q] (ones-column trick)
  U = O^T -> SBUF; Un = U[0:64] / r    (recip + partition broadcast)
  out[q,:] += Un_h.T @ Wp_h            accumulated over both heads in PSUM
"""

import os
import sys

import numpy as np

sys.path.insert(0, "/opt/trn_rl_repo")

import ml_dtypes
from contextlib import ExitStack

import concourse.bass as bass
import concourse.mybir as mybir
import concourse.tile as tile
from concourse import bacc
from concourse.bass_utils import run_bass_kernel_spmd

B, N, D, H, HS = 2, 2048, 512, 8, 64
NCORES = 8
BF16 = mybir.dt.bfloat16
FP32 = mybir.dt.float32
nbf16 = ml_dtypes.bfloat16

DC = D // 128  # 4 d-chunks
MC = N // 128  # 16 m-chunks
QH = 2  # q halves
QW = N // QH  # 1024 q per chunk


def build_nc(finalize=True):
    nc = bacc.Bacc()
    # x tensors arrive pre-transposed [D, N] and w tensors pre-swizzled to
    # the on-chip [128, dc*128] layout (host-side prep), so every input DMA
    # is a big contiguous transfer - no xbar transposes (which the tile
    # scheduler serializes against all other DMAs)
    xq = nc.dram_tensor("xq", [D, N], BF16, kind="ExternalInput")
    xk = nc.dram_tensor("xk", [D, N], BF16, kind="ExternalInput")
    xv = nc.dram_tensor("xv", [D, N], BF16, kind="ExternalInput")
    # all weights in one tensor = one DMA trigger: [p, 4, dc*128] holding
    # wq|wk|wv (swizzled) and wp
    wall = nc.dram_tensor("wall", [128, 4, DC * 128], BF16, kind="ExternalInput")
    out = nc.dram_tensor("out", [N, D], FP32, kind="ExternalOutput")

    with tile.TileContext(nc) as tc, ExitStack() as ctx:
        consts = ctx.enter_context(tc.tile_pool(name="consts", bufs=1))
        xt_pool = ctx.enter_context(tc.tile_pool(name="xt", bufs=1))
        proj_pool = ctx.enter_context(tc.tile_pool(name="proj", bufs=1))
        pt_pool = ctx.enter_context(tc.tile_pool(name="pt", bufs=16))
        u_pool = ctx.enter_context(tc.tile_pool(name="u", bufs=4))
        un_pool = ctx.enter_context(tc.tile_pool(name="un", bufs=4))
        rb_pool = ctx.enter_context(tc.tile_pool(name="rb", bufs=2))
        ob_pool = ctx.enter_context(tc.tile_pool(name="ob", bufs=3))
        psA = ctx.enter_context(tc.tile_pool(name="psA", bufs=2, space="PSUM"))
        psO = ctx.enter_context(tc.tile_pool(name="psO", bufs=2, space="PSUM"))

        # ---- weights: one DMA on the act HWDGE ring ----
        wall_s = consts.tile([128, 4, DC, 128], BF16, tag="wall_s")
        wq_s = wall_s[:, 0]
        wk_s = wall_s[:, 1]
        wv_s = wall_s[:, 2]
        wp_s = wall_s[:, 3].rearrange("p c h -> p (c h)")
        # Vnat: [128m, mc, head, 65]; col 64 = ones (rowsum trick)
        vnat = consts.tile([128, MC, 2, HS + 1], BF16, tag="vnat")
        # lhsT/rhs must share a base partition; the rowsum row lives at
        # partition HS, so put the ones row there too
        ones_row = consts.tile([HS + 1, HS], BF16, tag="ones_row")
        # scratch for PE warm-up filler matmuls (flips the HAM clock gate to
        # 2.4 GHz before the S stream starts, while waiting on the q DMA);
        # memset first so the PE can start as early as possible
        fill_sb = consts.tile([128, 1024], BF16, tag="fill_sb")
        nc.gpsimd.memset(fill_sb[:], 0.125)
        nc.gpsimd.memset(vnat[:, :, :, HS : HS + 1], 1.0)
        nc.gpsimd.memset(ones_row[HS : HS + 1, :], 1.0)
        # preload the Exp table during the preamble so the first real exp
        # doesn't pay the lazy ACT_TABLE_LOAD (~1.3us)
        tbl_warm = consts.tile([1, 1], BF16, tag="tbl_warm")
        nc.scalar.activation(
            tbl_warm[:], fill_sb[0:1, 0:1], mybir.ActivationFunctionType.Exp
        )

        def emit_fill(n, pool=None, tag="ps"):
            # n x 512-column matmuls with no data dependencies: PE activity
            # filler that holds the HAM clock gate at 8/8
            for _f in range(n):
                fps = (pool or psA).tile(
                    [128, 512], FP32, tag=tag, name="fill_ps"
                )
                nc.tensor.matmul(
                    fps[:], fill_sb[:, 0:128], fill_sb[:, 0:512],
                    start=True, stop=True,
                )

        def emit_ldw_fill(n):
            # PSUM-free PE-array activity (weight loads of a ready constant):
            # can never stall the queue, keeps the HAM clock gate warm
            # through thin stretches
            for _f in range(n):
                nc.tensor.ldweights(fill_sb[:, 0:128])

        # ---- X transposed: [128, dc, N] per tensor, plain chunked DMAs
        # (hosts sends X pre-transposed), k/v on the sync ring, q on the act
        # ring, ordered by first use so compute starts ~2us after the
        # engine preamble
        xts = {}
        for name in ("q", "k", "v"):
            xts[name] = xt_pool.tile(
                [128, DC, N], BF16, tag=f"xt_{name}", name=f"xt_{name}"
            )

        def emit_xload(eng, name, lo, hi):
            dram = {"q": xq, "k": xk, "v": xv}[name]
            eng.dma_start(
                out=xts[name][:, :, lo:hi],
                in_=dram.rearrange("(c p) n -> p c n", p=128)[:, :, lo:hi],
            )

        wall_r = wall.rearrange("p f (c h) -> p f c h", h=128)
        nc.scalar.dma_start(out=wall_s[:, 0:2], in_=wall_r[:, 0:2])
        emit_xload(nc.scalar, "q", 0, 1024)
        nc.scalar.dma_start(out=wall_s[:, 2:4], in_=wall_r[:, 2:4])
        emit_xload(nc.scalar, "q", 1024, 2048)
        emit_xload(nc.sync, "k", 0, 512)
        emit_xload(nc.sync, "k", 512, 1024)
        emit_xload(nc.sync, "v", 0, 1024)
        emit_xload(nc.sync, "k", 1024, 1536)
        emit_xload(nc.sync, "k", 1536, 2048)
        emit_xload(nc.sync, "v", 1024, 2048)

        # ---- projections ----
        wmap = {"q": wq_s, "k": wk_s}
        projT = {}
        for name in ("q", "k"):
            projT[name] = proj_pool.tile(
                [128, N], BF16, tag=f"projT_{name}", name=f"projT_{name}"
            )

        def emit_proj(name, lo, hi):
            # [2*HS, lo:hi] = sum_dc W[dc].T @ XT[dc, lo:hi]
            w = hi - lo
            ps = psA.tile([128, w], FP32, tag="ps", name="ps")
            for s0 in range(0, w, 512):
                sw = min(512, w - s0)
                for dc in range(DC):
                    nc.tensor.matmul(
                        ps[:, s0 : s0 + sw],
                        wmap[name][:, dc, :],
                        xts[name][:, dc, lo + s0 : lo + s0 + sw],
                        start=(dc == 0),
                        stop=(dc == DC - 1),
                    )
            nc.vector.tensor_copy(projT[name][:, lo:hi], ps[:])

        def emit_vnat(g):
            # natural-layout V for m-chunks 4g..4g+3: [128m, 128hs2] each
            ps = psA.tile([128, 512], FP32, tag="ps", name="vg_ps")
            for i in range(4):
                m0 = (4 * g + i) * 128
                for dc in range(DC):
                    nc.tensor.matmul(
                        ps[:, i * 128 : (i + 1) * 128],
                        xts["v"][:, dc, m0 : m0 + 128],
                        wv_s[:, dc, :],
                        start=(dc == 0),
                        stop=(dc == DC - 1),
                    )
            nc.vector.tensor_copy(
                vnat[:, 4 * g : 4 * (g + 1), :, 0:HS],
                ps[:].rearrange("p (m b c) -> p m b c", m=4, b=2),
            )

        # pre-loop: HAM warm-up fillers run while the k/q DMAs are still in
        # flight (PE busy 3.4us+ flips the clock gate), then just enough
        # projection for the first S chunks, with more fillers woven in to
        # bridge the DMA-gated stretch up to the first S without idling
        for _f in range(15):
            fps = psO.tile([HS, QW], FP32, tag="o", name="fill_ps")
            for sl in range(QW // 512):
                nc.tensor.matmul(
                    fps[:, sl * 512 : (sl + 1) * 512],
                    fill_sb[:, 0:HS],
                    fill_sb[:, sl * 512 : (sl + 1) * 512],
                    start=True,
                    stop=True,
                )
        emit_proj("k", 0, 512)
        emit_proj("q", 0, 1024)

        # deferred work, paced into the first m-loop at given (mc, hh) slots
        # so the PE never stalls on a DMA that hasn't landed yet
        deferred = {
            (1, 0): lambda: emit_proj("k", 512, 1024),
            (2, 0): lambda: emit_vnat(0),
            (3, 0): lambda: emit_proj("k", 1024, 1536),
            (4, 0): lambda: emit_proj("k", 1536, 2048),
            (5, 0): lambda: emit_vnat(1),
            (6, 0): lambda: emit_proj("q", 1024, 2048),
            (8, 0): lambda: emit_vnat(2),
            (9, 0): lambda: emit_vnat(3),
        }
        trickle = []  # qh0's normalization/final chain, run during qh1 loop
        carry = []  # qh0's last PVs, replayed through qh1's ramp units

        # attention + output projection - both heads' m-loops
        # interleaved so the PE stream stays dense (holds HAM warm)
        qt2, kt2 = projT["q"], projT["k"]
        for qh in range(QH):
            un2 = un_pool.tile([128, QW], BF16, tag="un")
            o_ps = {}

            def alloc_o():
                for hh in range(2):
                    o_ps[hh] = psO.tile(
                        [HS + 1, QW], FP32, tag="o", name=f"o_ps{hh}"
                    )



            def pv(hh, j, p_sb, o_ps=o_ps):
                for sl in range(QW // 512):
                    nc.tensor.matmul(
                        o_ps[hh][:, sl * 512 : (sl + 1) * 512],
                        vnat[:, j, hh, :],
                        p_sb[:, sl * 512 : (sl + 1) * 512],
                        start=(j == 0),
                        stop=(j == MC - 1),
                    )

            pend = []
            for mc in range(MC):
                for hh in range(2):
                    hs0 = HS * hh
                    s_ps = psA.tile([128, QW], FP32, tag="ps", name="s_ps")
                    for sl in range(QW // 512):
                        nc.tensor.matmul(
                            s_ps[:, sl * 512 : (sl + 1) * 512],
                            kt2[hs0 : hs0 + HS, mc * 128 : (mc + 1) * 128],
                            qt2[
                                hs0 : hs0 + HS,
                                qh * QW + sl * 512 : qh * QW + (sl + 1) * 512,
                            ],
                            start=True,
                            stop=True,
                        )
                    p_sb = pt_pool.tile([128, QW], BF16, tag="p", name="p_sb")
                    nc.scalar.activation(
                        p_sb[:], s_ps[:], mybir.ActivationFunctionType.Exp
                    )
                    if qh == 0:
                        if mc < 3:
                            # the PE is s_ps-slot-limited while the ACT
                            # pipeline ramps: pad its duty so the clock
                            # gate holds 8/8 (o_ps not yet allocated, so
                            # the psO slots are free)
                            emit_fill(1, pool=psO, tag="o")
                        if (mc, hh) == (3, 0):
                            alloc_o()
                        fn = deferred.pop((mc, hh), None)
                        if fn is not None:
                            fn()
                    else:
                        if carry:
                            # replay a ready PV from the previous half:
                            # keeps PE duty high through this half's ramp
                            cpv, e = carry.pop(0)
                            cpv(*e)
                        if trickle and not carry:
                            trickle.pop(0)()
                        if (mc, hh) == (3, 0):
                            alloc_o()
                    pend.append((hh, mc, p_sb))
                    if qh == 0:
                        # hold the last 8 PVs; they replay through the next
                        # half's ramp units to keep PE duty high
                        lag = 6 if mc < 14 else 8
                    else:
                        lag = 9 if mc < 7 else (6 if mc < MC - 1 else 2)
                    while len(pend) > lag:
                        pv(*pend.pop(0))

            def emit_ucopy(hh, o_ps_=None, ueng=None, lo=0, w=QW):
                # PSUM -> SBUF stage; frees the o_ps slot for the next
                # q-half's PV accumulation
                o_ps_ = o_ps_ if o_ps_ is not None else o_ps
                u = u_pool.tile([HS + 1, w], BF16, tag="u", name="u")
                if ueng is None:
                    nc.vector.tensor_copy(u[:], o_ps_[hh][:, lo : lo + w])
                else:
                    ueng.copy(u[:], o_ps_[hh][:, lo : lo + w])
                return u

            def emit_norm(hh, u, un2_, lo=0, w=QW):
                # broadcast row sums r to 64 partitions via ones.T @ r
                # (rb reuses a freed o_ps slot)
                rb_ps = psO.tile([HS, w], FP32, tag="o", name="rb_ps")
                for sl in range(w // 512):
                    nc.tensor.matmul(
                        rb_ps[:, sl * 512 : (sl + 1) * 512],
                        ones_row[HS : HS + 1, :],
                        u[HS : HS + 1, sl * 512 : (sl + 1) * 512],
                        start=True,
                        stop=True,
                    )
                rb = rb_pool.tile([HS, w], FP32, tag="rb", name="rb")
                nc.vector.reciprocal_approx_fast(rb[:], rb_ps[:])
                nc.vector.tensor_mul(
                    un2_[HS * hh : HS * hh + HS, lo : lo + w], u[0:HS, :], rb[:]
                )

            def emit_uchain(hh, o_ps_, un2_):
                u = emit_ucopy(hh, o_ps_)
                emit_norm(hh, u, un2_)

            def emit_final(qh_, un2_, c, act_copy=False):
                f_ps = psA.tile([128, D], FP32, tag="ps", name="f_ps")
                nc.tensor.matmul(
                    f_ps[:],
                    un2_[:, c * 128 : (c + 1) * 128],
                    wp_s[:],
                    start=True,
                    stop=True,
                )
                ob = ob_pool.tile([128, D], FP32, tag="ob", name="ob")
                if act_copy:
                    nc.scalar.copy(ob[:], f_ps[:])
                    deng = nc.scalar
                else:
                    nc.vector.tensor_copy(ob[:], f_ps[:])
                    deng = nc.sync
                deng.dma_start(
                    out=out[qh_ * QW + c * 128 : qh_ * QW + (c + 1) * 128, :],
                    in_=ob[:],
                )

            if qh < QH - 1:
                carry.extend((pv, e) for e in pend)
                pend = []
                for hh in range(2):
                    trickle.append(
                        lambda hh_=hh, o_=o_ps, u_=un2: emit_uchain(hh_, o_, u_)
                    )
                for c in range(QW // 128):
                    trickle.append(
                        lambda qh_=qh, un2_=un2, c_=c: emit_final(qh_, un2_, c_)
                    )
            else:
                # staggered tail: both last PVs first on the PE queue, then
                # per-512-column chains with h1's u copies on the (now idle)
                # ACT engine; finals start as soon as their column's chains
                # are done; copies and out-DMAs alternate ACT/sync rings
                # h0's last PVs first so its u copies (DVE) overlap h1's
                # final PVs on the PE; then h1's copies on ACT
                for e in [e for e in pend if e[0] == 0]:
                    pv(*e)
                uas = [emit_ucopy(0, lo=c * 512, w=512) for c in range(2)]
                for e in [e for e in pend if e[0] == 1]:
                    pv(*e)
                ubs = [
                    emit_ucopy(1, ueng=nc.scalar, lo=c * 512, w=512)
                    for c in range(2)
                ]
                us = list(zip(uas, ubs))
                for col in range(2):
                    lo = col * 512
                    ua, ub = us[col]
                    emit_norm(0, ua, un2, lo=lo, w=512)
                    emit_norm(1, ub, un2, lo=lo, w=512)
                    for c in range(4 * col, 4 * col + 4):
                        emit_final(qh, un2, c, act_copy=(c % 2 == 1))
    if finalize:
        nc.finalize()
    return nc


_NC_CACHE = None


def _get_nc():
    global _NC_CACHE
    if _NC_CACHE is None:
        _NC_CACHE = build_nc()
    return _NC_CACHE


def make_in_maps(inputs):
    query = np.asarray(inputs["query"], np.float32)
    key = np.asarray(inputs["key"], np.float32)
    value = np.asarray(inputs["value"], np.float32)
    Wq = np.asarray(inputs["Wq"], np.float32) / np.sqrt(np.float32(HS))
    Wk = np.asarray(inputs["Wk"], np.float32)
    Wv = np.asarray(inputs["Wv"], np.float32)
    Wp = np.asarray(inputs["Wp"], np.float32)

    def wprep(w, h0):
        # [D, 128] head-pair matrix -> on-chip [128, dc*128] layout where
        # row p, col (c*128+h) holds w[c*128+p, h]
        ww = np.concatenate([w[h0], w[h0 + 1]], axis=1)  # [D, 128]
        return ww.reshape(DC, 128, 128).transpose(1, 0, 2).reshape(128, DC * 128)

    xqT = [np.ascontiguousarray(query[b].T).astype(nbf16) for b in range(B)]
    xkT = [np.ascontiguousarray(key[b].T).astype(nbf16) for b in range(B)]
    xvT = [np.ascontiguousarray(value[b].T).astype(nbf16) for b in range(B)]

    in_maps = []
    for c in range(NCORES):
        b = c // 4
        h0 = 2 * (c % 4)
        wall = np.stack(
            [
                wprep(Wq, h0),
                wprep(Wk, h0),
                wprep(Wv, h0),
                np.concatenate([Wp[h0], Wp[h0 + 1]], axis=0),
            ],
            axis=1,
        )  # [128, 4, dc*128]
        in_maps.append(
            {
                "xq": xqT[b],
                "xk": xkT[b],
                "xv": xvT[b],
                "wall": np.ascontiguousarray(wall).astype(nbf16),
            }
        )
    return in_maps


def kernel(query, key, value, Wq, Wk, Wv, Wp):
    in_maps = make_in_maps(
        dict(query=query, key=key, value=value, Wq=Wq, Wk=Wk, Wv=Wv, Wp=Wp)
    )
    nc = _get_nc()
    res = run_bass_kernel_spmd(nc, in_maps, list(range(NCORES)))
    out = np.zeros((B, N, D), np.float32)
    for c in range(NCORES):
        out[c // 4] += np.asarray(res.results[c]["out"], np.float32)
    return out


if __name__ == "__main__":
    d = np.load("/root/problem/work/ref.npz")
    got = kernel(
        d["query"], d["key"], d["value"], d["Wq"], d["Wk"], d["Wv"], d["Wp"]
    )
    exp = d["expected"]
    rel = np.linalg.norm(got - exp) / np.linalg.norm(exp)
    print("Relative error:", rel)


# revision 75
# speedup vs baseline: 1.1964x; 1.0032x over previous
"""Multi-head attention Trainium2 kernel, 8-core SPMD.

Sharding: 16 (batch, head) pairs over 8 cores -> each core computes 2 heads
of one batch and returns a partial [N, D] output; host sums 4 partials per
batch.

Per-core dataflow (all layouts transposed, q/m on free dims so softmax'
normalization can be deferred):
  XT = dma-transpose(x)                [D, N]  bf16, chunked + split across
                                       both HWDGE rings (sync + act) so the
                                       first projection starts ~4us in
  QT/KT = W.T @ XT                     [2*HS, N] per head pair (scale folded
                                       into Wq on host)
  Vnat  = XT_chunk.T @ Wv              [m, 2*HS] per m-chunk (natural layout
                                       directly; no PE transposes)
  S^T[m,q] = KT_h.T @ QT_h             PSUM fp32, per m-chunk of 128
  P^T = exp(S^T)                       ACT, -> SBUF bf16 (no max subtraction:
                                       logits are O(6) by construction)
  O^T[65,q] = [V_h | 1].T @ P^T        PSUM accumulate over m; row 64 = row
                                       sums r[q] (ones-column trick)
  U = O^T -> SBUF; Un = U[0:64] / r    (recip + partition broadcast)
  out[q,:] += Un_h.T @ Wp_h            accumulated over both heads in PSUM
"""

import os
import sys

import numpy as np

sys.path.insert(0, "/opt/trn_rl_repo")

import ml_dtypes
from contextlib import ExitStack

import concourse.bass as bass
import concourse.mybir as mybir
import concourse.tile as tile
from concourse import bacc
from concourse.bass_utils import run_bass_kernel_spmd

B, N, D, H, HS = 2, 2048, 512, 8, 64
NCORES = 8
BF16 = mybir.dt.bfloat16
FP32 = mybir.dt.float32
nbf16 = ml_dtypes.bfloat16

DC = D // 128  # 4 d-chunks
MC = N // 128  # 16 m-chunks
QH = 2  # q halves
QW = N // QH  # 1024 q per chunk


def build_nc(finalize=True):
    nc = bacc.Bacc()
    # x tensors arrive pre-transposed [D, N] and w tensors pre-swizzled to
    # the on-chip [128, dc*128] layout (host-side prep), so every input DMA
    # is a big contiguous transfer - no xbar transposes (which the tile
    # scheduler serializes against all other DMAs)
    xq = nc.dram_tensor("xq", [D, N], BF16, kind="ExternalInput")
    xk = nc.dram_tensor("xk", [D, N], BF16, kind="ExternalInput")
    xv = nc.dram_tensor("xv", [D, N], BF16, kind="ExternalInput")
    # all weights in one tensor = one DMA trigger: [p, 4, dc*128] holding
    # wq|wk|wv (swizzled) and wp
    wall = nc.dram_tensor("wall", [128, 4, DC * 128], BF16, kind="ExternalInput")
    out = nc.dram_tensor("out", [N, D], FP32, kind="ExternalOutput")

    with tile.TileContext(nc) as tc, ExitStack() as ctx:
        consts = ctx.enter_context(tc.tile_pool(name="consts", bufs=1))
        xt_pool = ctx.enter_context(tc.tile_pool(name="xt", bufs=1))
        proj_pool = ctx.enter_context(tc.tile_pool(name="proj", bufs=1))
        pt_pool = ctx.enter_context(tc.tile_pool(name="pt", bufs=16))
        u_pool = ctx.enter_context(tc.tile_pool(name="u", bufs=4))
        un_pool = ctx.enter_context(tc.tile_pool(name="un", bufs=4))
        rb_pool = ctx.enter_context(tc.tile_pool(name="rb", bufs=2))
        ob_pool = ctx.enter_context(tc.tile_pool(name="ob", bufs=3))
        psA = ctx.enter_context(tc.tile_pool(name="psA", bufs=2, space="PSUM"))
        psO = ctx.enter_context(tc.tile_pool(name="psO", bufs=2, space="PSUM"))

        # ---- weights: one DMA on the act HWDGE ring ----
        wall_s = consts.tile([128, 4, DC, 128], BF16, tag="wall_s")
        wq_s = wall_s[:, 0]
        wk_s = wall_s[:, 1]
        wv_s = wall_s[:, 2]
        wp_s = wall_s[:, 3].rearrange("p c h -> p (c h)")
        # Vnat: [128m, mc, head, 65]; col 64 = ones (rowsum trick)
        vnat = consts.tile([128, MC, 2, HS + 1], BF16, tag="vnat")
        # lhsT/rhs must share a base partition; the rowsum row lives at
        # partition HS, so put the ones row there too
        ones_row = consts.tile([HS + 1, HS], BF16, tag="ones_row")
        # scratch for PE warm-up filler matmuls (flips the HAM clock gate to
        # 2.4 GHz before the S stream starts, while waiting on the q DMA);
        # memset first so the PE can start as early as possible
        fill_sb = consts.tile([128, 1024], BF16, tag="fill_sb")
        nc.gpsimd.memset(fill_sb[:], 0.125)
        nc.gpsimd.memset(vnat[:, :, :, HS : HS + 1], 1.0)
        nc.gpsimd.memset(ones_row[HS : HS + 1, :], 1.0)
        # preload the Exp table during the preamble so the first real exp
        # doesn't pay the lazy ACT_TABLE_LOAD (~1.3us)
        tbl_warm = consts.tile([1, 1], BF16, tag="tbl_warm")
        nc.scalar.activation(
            tbl_warm[:], fill_sb[0:1, 0:1], mybir.ActivationFunctionType.Exp
        )

        def emit_fill(n, pool=None, tag="ps"):
            # n x 512-column matmuls with no data dependencies: PE activity
            # filler that holds the HAM clock gate at 8/8
            for _f in range(n):
                fps = (pool or psA).tile(
                    [128, 512], FP32, tag=tag, name="fill_ps"
                )
                nc.tensor.matmul(
                    fps[:], fill_sb[:, 0:128], fill_sb[:, 0:512],
                    start=True, stop=True,
                )

        def emit_ldw_fill(n):
            # PSUM-free PE-array activity (weight loads of a ready constant):
            # can never stall the queue, keeps the HAM clock gate warm
            # through thin stretches
            for _f in range(n):
                nc.tensor.ldweights(fill_sb[:, 0:128])

        # ---- X transposed: [128, dc, N] per tensor, plain chunked DMAs
        # (hosts sends X pre-transposed), k/v on the sync ring, q on the act
        # ring, ordered by first use so compute starts ~2us after the
        # engine preamble
        xts = {}
        for name in ("q", "k", "v"):
            xts[name] = xt_pool.tile(
                [128, DC, N], BF16, tag=f"xt_{name}", name=f"xt_{name}"
            )

        def emit_xload(eng, name, lo, hi):
            dram = {"q": xq, "k": xk, "v": xv}[name]
            eng.dma_start(
                out=xts[name][:, :, lo:hi],
                in_=dram.rearrange("(c p) n -> p c n", p=128)[:, :, lo:hi],
            )

        wall_r = wall.rearrange("p f (c h) -> p f c h", h=128)
        nc.scalar.dma_start(out=wall_s[:, 0:2], in_=wall_r[:, 0:2])
        emit_xload(nc.scalar, "q", 0, 1024)
        nc.scalar.dma_start(out=wall_s[:, 2:4], in_=wall_r[:, 2:4])
        emit_xload(nc.scalar, "q", 1024, 2048)
        emit_xload(nc.sync, "k", 0, 512)
        emit_xload(nc.sync, "k", 512, 1024)
        emit_xload(nc.sync, "v", 0, 1024)
        emit_xload(nc.sync, "k", 1024, 1536)
        emit_xload(nc.sync, "k", 1536, 2048)
        emit_xload(nc.sync, "v", 1024, 2048)

        # ---- projections ----
        wmap = {"q": wq_s, "k": wk_s}
        projT = {}
        for name in ("q", "k"):
            projT[name] = proj_pool.tile(
                [128, N], BF16, tag=f"projT_{name}", name=f"projT_{name}"
            )

        def emit_proj(name, lo, hi):
            # [2*HS, lo:hi] = sum_dc W[dc].T @ XT[dc, lo:hi]
            w = hi - lo
            ps = psA.tile([128, w], FP32, tag="ps", name="ps")
            for s0 in range(0, w, 512):
                sw = min(512, w - s0)
                for dc in range(DC):
                    nc.tensor.matmul(
                        ps[:, s0 : s0 + sw],
                        wmap[name][:, dc, :],
                        xts[name][:, dc, lo + s0 : lo + s0 + sw],
                        start=(dc == 0),
                        stop=(dc == DC - 1),
                    )
            nc.vector.tensor_copy(projT[name][:, lo:hi], ps[:])

        def emit_vnat(g):
            # natural-layout V for m-chunks 4g..4g+3: [128m, 128hs2] each
            ps = psA.tile([128, 512], FP32, tag="ps", name="vg_ps")
            for i in range(4):
                m0 = (4 * g + i) * 128
                for dc in range(DC):
                    nc.tensor.matmul(
                        ps[:, i * 128 : (i + 1) * 128],
                        xts["v"][:, dc, m0 : m0 + 128],
                        wv_s[:, dc, :],
                        start=(dc == 0),
                        stop=(dc == DC - 1),
                    )
            nc.vector.tensor_copy(
                vnat[:, 4 * g : 4 * (g + 1), :, 0:HS],
                ps[:].rearrange("p (m b c) -> p m b c", m=4, b=2),
            )

        # pre-loop: HAM warm-up fillers run while the k/q DMAs are still in
        # flight (PE busy 3.4us+ flips the clock gate), then just enough
        # projection for the first S chunks, with more fillers woven in to
        # bridge the DMA-gated stretch up to the first S without idling
        for _f in range(15):
            fps = psO.tile([HS, QW], FP32, tag="o", name="fill_ps")
            for sl in range(QW // 512):
                nc.tensor.matmul(
                    fps[:, sl * 512 : (sl + 1) * 512],
                    fill_sb[:, 0:HS],
                    fill_sb[:, sl * 512 : (sl + 1) * 512],
                    start=True,
                    stop=True,
                )
        emit_proj("k", 0, 512)
        emit_proj("q", 0, 1024)

        # deferred work, paced into the first m-loop at given (mc, hh) slots
        # so the PE never stalls on a DMA that hasn't landed yet
        deferred = {
            (1, 0): lambda: emit_proj("k", 512, 1024),
            (2, 0): lambda: emit_vnat(0),
            (3, 0): lambda: emit_proj("k", 1024, 1536),
            (4, 0): lambda: emit_proj("k", 1536, 2048),
            (5, 0): lambda: emit_vnat(1),
            (6, 0): lambda: emit_proj("q", 1024, 2048),
            (8, 0): lambda: emit_vnat(2),
            (9, 0): lambda: emit_vnat(3),
        }
        trickle = []  # qh0's normalization/final chain, run during qh1 loop
        carry = []  # qh0's last PVs, replayed through qh1's ramp units

        # attention + output projection - both heads' m-loops
        # interleaved so the PE stream stays dense (holds HAM warm)
        qt2, kt2 = projT["q"], projT["k"]
        for qh in range(QH):
            un2 = un_pool.tile([128, QW], BF16, tag="un")
            o_ps = {}

            def alloc_o():
                for hh in range(2):
                    o_ps[hh] = psO.tile(
                        [HS + 1, QW], FP32, tag="o", name=f"o_ps{hh}"
                    )



            def pv(hh, j, p_sb, o_ps=o_ps):
                for sl in range(QW // 512):
                    nc.tensor.matmul(
                        o_ps[hh][:, sl * 512 : (sl + 1) * 512],
                        vnat[:, j, hh, :],
                        p_sb[:, sl * 512 : (sl + 1) * 512],
                        start=(j == 0),
                        stop=(j == MC - 1),
                    )

            pend = []
            for mc in range(MC):
                for hh in range(2):
                    hs0 = HS * hh
                    s_ps = psA.tile([128, QW], FP32, tag="ps", name="s_ps")
                    for sl in range(QW // 512):
                        nc.tensor.matmul(
                            s_ps[:, sl * 512 : (sl + 1) * 512],
                            kt2[hs0 : hs0 + HS, mc * 128 : (mc + 1) * 128],
                            qt2[
                                hs0 : hs0 + HS,
                                qh * QW + sl * 512 : qh * QW + (sl + 1) * 512,
                            ],
                            start=True,
                            stop=True,
                        )
                    p_sb = pt_pool.tile([128, QW], BF16, tag="p", name="p_sb")
                    nc.scalar.activation(
                        p_sb[:], s_ps[:], mybir.ActivationFunctionType.Exp
                    )
                    # PV drain first: the extras below allocate from the
                    # same 2-slot PSUM ring as s_ps and so wait on a prior
                    # exp; queued after the PVs, that wait overlaps real PV
                    # work instead of delaying the next S tile
                    if (mc, hh) == (3, 0):
                        alloc_o()
                    pend.append((hh, mc, p_sb))
                    if qh == 0:
                        lag = 6 if mc < 14 else 8
                    else:
                        lag = 9 if mc < 7 else (6 if mc < MC - 1 else 2)
                    while len(pend) > lag:
                        pv(*pend.pop(0))
                    if qh == 0:
                        if mc < 3:
                            # the PE is s_ps-slot-limited while the ACT
                            # pipeline ramps: pad its duty so the clock
                            # gate holds 8/8 (o_ps not yet allocated, so
                            # the psO slots are free)
                            emit_fill(1, pool=psO, tag="o")
                        fn = deferred.pop((mc, hh), None)
                        if fn is not None:
                            fn()
                    else:
                        if carry:
                            # replay a ready PV from the previous half:
                            # keeps PE duty high through this half's ramp
                            cpv, e = carry.pop(0)
                            cpv(*e)
                        if trickle and not carry:
                            trickle.pop(0)()

            def emit_ucopy(hh, o_ps_=None, ueng=None, lo=0, w=QW):
                # PSUM -> SBUF stage; frees the o_ps slot for the next
                # q-half's PV accumulation
                o_ps_ = o_ps_ if o_ps_ is not None else o_ps
                u = u_pool.tile([HS + 1, w], BF16, tag="u", name="u")
                if ueng is None:
                    nc.vector.tensor_copy(u[:], o_ps_[hh][:, lo : lo + w])
                else:
                    ueng.copy(u[:], o_ps_[hh][:, lo : lo + w])
                return u

            def emit_norm(hh, u, un2_, lo=0, w=QW):
                # broadcast row sums r to 64 partitions via ones.T @ r
                # (rb reuses a freed o_ps slot)
                rb_ps = psO.tile([HS, w], FP32, tag="o", name="rb_ps")
                for sl in range(w // 512):
                    nc.tensor.matmul(
                        rb_ps[:, sl * 512 : (sl + 1) * 512],
                        ones_row[HS : HS + 1, :],
                        u[HS : HS + 1, sl * 512 : (sl + 1) * 512],
                        start=True,
                        stop=True,
                    )
                rb = rb_pool.tile([HS, w], FP32, tag="rb", name="rb")
                nc.vector.reciprocal_approx_fast(rb[:], rb_ps[:])
                nc.vector.tensor_mul(
                    un2_[HS * hh : HS * hh + HS, lo : lo + w], u[0:HS, :], rb[:]
                )

            def emit_uchain(hh, o_ps_, un2_):
                u = emit_ucopy(hh, o_ps_)
                emit_norm(hh, u, un2_)

            def emit_final(qh_, un2_, c, act_copy=False):
                f_ps = psA.tile([128, D], FP32, tag="ps", name="f_ps")
                nc.tensor.matmul(
                    f_ps[:],
                    un2_[:, c * 128 : (c + 1) * 128],
                    wp_s[:],
                    start=True,
                    stop=True,
                )
                ob = ob_pool.tile([128, D], FP32, tag="ob", name="ob")
                if act_copy:
                    nc.scalar.copy(ob[:], f_ps[:])
                    deng = nc.scalar
                else:
                    nc.vector.tensor_copy(ob[:], f_ps[:])
                    deng = nc.sync
                deng.dma_start(
                    out=out[qh_ * QW + c * 128 : qh_ * QW + (c + 1) * 128, :],
                    in_=ob[:],
                )

            if qh < QH - 1:
                carry.extend((pv, e) for e in pend)
                pend = []
                for hh in range(2):
                    trickle.append(
                        lambda hh_=hh, o_=o_ps, u_=un2: emit_uchain(hh_, o_, u_)
                    )
                for c in range(QW // 128):
                    trickle.append(
                        lambda qh_=qh, un2_=un2, c_=c: emit_final(qh_, un2_, c_)
                    )
            else:
                # staggered tail: both last PVs first on the PE queue, then
                # per-512-column chains with h1's u copies on the (now idle)
                # ACT engine; finals start as soon as their column's chains
                # are done; copies and out-DMAs alternate ACT/sync rings
                # h0's last PVs first so its u copies (DVE) overlap h1's
                # final PVs on the PE; then h1's copies on ACT
                for e in [e for e in pend if e[0] == 0]:
                    pv(*e)
                uas = [emit_ucopy(0, lo=c * 512, w=512) for c in range(2)]
                for e in [e for e in pend if e[0] == 1]:
                    pv(*e)
                ubs = [
                    emit_ucopy(1, ueng=nc.scalar, lo=c * 512, w=512)
                    for c in range(2)
                ]
                us = list(zip(uas, ubs))
                for col in range(2):
                    lo = col * 512
                    ua, ub = us[col]
                    emit_norm(0, ua, un2, lo=lo, w=512)
                    emit_norm(1, ub, un2, lo=lo, w=512)
                    for c in range(4 * col, 4 * col + 4):
                        emit_final(qh, un2, c, act_copy=(c % 2 == 1))
    if finalize:
        nc.finalize()
    return nc


_NC_CACHE = None


def _get_nc():
    global _NC_CACHE
    if _NC_CACHE is None:
        _NC_CACHE = build_nc()
    return _NC_CACHE


def make_in_maps(inputs):
    query = np.asarray(inputs["query"], np.float32)
    key = np.asarray(inputs["key"], np.float32)
    value = np.asarray(inputs["value"], np.float32)
    Wq = np.asarray(inputs["Wq"], np.float32) / np.sqrt(np.float32(HS))
    Wk = np.asarray(inputs["Wk"], np.float32)
    Wv = np.asarray(inputs["Wv"], np.float32)
    Wp = np.asarray(inputs["Wp"], np.float32)

    def wprep(w, h0):
        # [D, 128] head-pair matrix -> on-chip [128, dc*128] layout where
        # row p, col (c*128+h) holds w[c*128+p, h]
        ww = np.concatenate([w[h0], w[h0 + 1]], axis=1)  # [D, 128]
        return ww.reshape(DC, 128, 128).transpose(1, 0, 2).reshape(128, DC * 128)

    xqT = [np.ascontiguousarray(query[b].T).astype(nbf16) for b in range(B)]
    xkT = [np.ascontiguousarray(key[b].T).astype(nbf16) for b in range(B)]
    xvT = [np.ascontiguousarray(value[b].T).astype(nbf16) for b in range(B)]

    in_maps = []
    for c in range(NCORES):
        b = c // 4
        h0 = 2 * (c % 4)
        wall = np.stack(
            [
                wprep(Wq, h0),
                wprep(Wk, h0),
                wprep(Wv, h0),
                np.concatenate([Wp[h0], Wp[h0 + 1]], axis=0),
            ],
            axis=1,
        )  # [128, 4, dc*128]
        in_maps.append(
            {
                "xq": xqT[b],
                "xk": xkT[b],
                "xv": xvT[b],
                "wall": np.ascontiguousarray(wall).astype(nbf16),
            }
        )
    return in_maps


def kernel(query, key, value, Wq, Wk, Wv, Wp):
    in_maps = make_in_maps(
        dict(query=query, key=key, value=value, Wq=Wq, Wk=Wk, Wv=Wv, Wp=Wp)
    )
    nc = _get_nc()
    res = run_bass_kernel_spmd(nc, in_maps, list(range(NCORES)))
    out = np.zeros((B, N, D), np.float32)
    for c in range(NCORES):
        out[c // 4] += np.asarray(res.results[c]["out"], np.float32)
    return out


if __name__ == "__main__":
    d = np.load("/root/problem/work/ref.npz")
    got = kernel(
        d["query"], d["key"], d["value"], d["Wq"], d["Wk"], d["Wv"], d["Wp"]
    )
    exp = d["expected"]
    rel = np.linalg.norm(got - exp) / np.linalg.norm(exp)
    print("Relative error:", rel)
